# revision 1
# baseline (speedup 1.0000x reference)
"""Multi-head attention (B=4, S=2048, d_model=1024, 16 heads x 64) on 8 trn2 cores.

Sharding: tensor-parallel over heads -- each core owns 2 heads (128 of the
1024 q/k/v dims and 128 rows of Wo's input dim). Each core computes a
partial output projection yT_c [1024, 8192] (fp16); the host sums the 8
partials and adds the combined bias (bo + bv @ Wo.T -- the V bias commutes
through softmax-weighted averaging since the weights sum to 1).

v2 design vs baseline:
- all matmuls in fp16 (1 cycle/row on the PE at any p-state, vs fp32r's
  4x penalty under 256-wide moving tiles and fp32's 4x always)
- softmax exp splits between the scalar (Act) engine (true EXP table) and a
  DVE Schraudolph bit-trick exp (int16 bits = scale*psum + offset, bitcast
  to fp16; ~0.1% weight error, washes out in softmax normalization)
- softmax row sums come free from a ones-column in V; normalization uses
  reciprocal_approx_fast + a rank-1 PE broadcast onto the [64, q] output
- exp applies a constant shift C (exp(s - C)): uniform factor per row that
  cancels in normalization, keeping fp16 att weights in a good range
"""

import numpy as np
import ml_dtypes

import concourse.bass as bass
import concourse.mybir as mybir
from concourse import bacc
from concourse.tile import TileContext
from concourse.masks import make_identity
from concourse.bass_utils import run_bass_kernel_spmd

N_HEAD = 16
D_HEAD = 64
D_MODEL = N_HEAD * D_HEAD  # 1024
B, S = 4, 2048
N_CORES = 8
HPC = N_HEAD // N_CORES  # heads per core = 2
HD = HPC * D_HEAD        # per-core head dims = 128

F32 = mybir.dt.float32
F32R = mybir.dt.float32r
F16 = mybir.dt.float16
I16 = mybir.dt.int16
AF = mybir.ActivationFunctionType
ALU = mybir.AluOpType
F16NP = np.float16

# ---- scale constants ----
SQ = 8.0          # q/k pre-scale: psum score = SQ^2 * 8 * s = 512 * s
C_SHIFT = 2.6     # att = exp(s - C_SHIFT); cancels in normalization
L2E = 1.4426950408889634
A16 = 1024.0 * L2E / 512.0               # DVE exp: bits = psum*A16 + B16
B16 = 15360.0 - 1024.0 * L2E * C_SHIFT - 60.0  # -60: Schraudolph correction
ACT_SCALE = 1.0 / 512.0

_TRACE = False  # test harness can flip this for profiling

# fraction of exp tiles computed on DVE instead of Act: idx % DVE_DEN < DVE_NUM
DVE_NUM = 0
DVE_DEN = 5


def build_mha(b=B, s=S, dve_num=DVE_NUM, dve_den=DVE_DEN):
    """Build the per-core Bass program (SPMD; all cores run this)."""
    P = 128
    tok = b * s
    dmc = D_MODEL // P        # 8 contraction chunks
    n_tc = s // 512           # 512-token chunks per batch
    n_kt = s // P             # k chunks per batch
    n_qh = s // 1024          # q halves per batch

    nc = bacc.Bacc("TRN2", target_bir_lowering=False, debug=False)

    xT = nc.dram_tensor("xT", [P, dmc, tok], F16, kind="ExternalInput")
    wq = nc.dram_tensor("wq", [P, dmc, HD], F16, kind="ExternalInput")
    wk = nc.dram_tensor("wk", [P, dmc, HD], F16, kind="ExternalInput")
    wv = nc.dram_tensor("wv", [P, dmc, HD], F16, kind="ExternalInput")
    wo = nc.dram_tensor("wo", [HD, D_MODEL], F16, kind="ExternalInput")
    bq = nc.dram_tensor("bq", [HD, 1], F32, kind="ExternalInput")
    bk = nc.dram_tensor("bk", [HD, 1], F32, kind="ExternalInput")
    yT = nc.dram_tensor("yT", [D_MODEL, tok], F16, kind="ExternalOutput")

    exp_idx = 0  # running exp-tile index for Act/DVE split

    with TileContext(nc) as tc:
        with (
            nc.allow_low_precision(reason="fp16 tiles feed the PE by design"),
            tc.tile_pool(name="const", bufs=1) as const,
            tc.tile_pool(name="xin", bufs=4) as xin,
            tc.tile_pool(name="stg", bufs=3) as stg,
            tc.tile_pool(name="vp", bufs=3) as vp,
            tc.tile_pool(name="att", bufs=8) as attp,
            tc.tile_pool(name="at16", bufs=2) as at16p,
            tc.tile_pool(name="out", bufs=4) as outp,
            tc.tile_pool(name="smal", bufs=4) as smal,
            tc.tile_pool(name="psA", bufs=2, space="PSUM") as psA,
            tc.tile_pool(name="psS", bufs=2, space="PSUM") as psS,
            tc.tile_pool(name="psO", bufs=1, space="PSUM") as psO,
        ):
            # ---- constants (resident) ----
            wq_sb = const.tile([P, dmc, HD], F16)
            wk_sb = const.tile([P, dmc, HD], F16)
            wv_sb = const.tile([P, dmc, HD], F16)
            wo_sb = const.tile([HD, D_MODEL], F16)
            bq_sb = const.tile([HD, 1], F32)
            bk_sb = const.tile([HD, 1], F32)
            ident16 = const.tile([P, P], F16)
            make_identity(nc, ident16[:])
            ones_a = const.tile([1, D_HEAD], F32R)
            onesf = const.tile([1, D_HEAD], F32)
            nc.vector.memset(onesf[:], 1.0)
            nc.vector.tensor_copy(ones_a[:], onesf[:])
            ebias = const.tile([P, 1], F32)
            nc.vector.memset(ebias[:], -C_SHIFT)
            nc.sync.dma_start(wq_sb[:], wq[:, :, :])
            nc.sync.dma_start(wk_sb[:], wk[:, :, :])
            nc.sync.dma_start(wv_sb[:], wv[:, :, :])
            nc.sync.dma_start(wo_sb[:], wo[:, :])
            nc.sync.dma_start(bq_sb[:], bq[:, :])
            nc.sync.dma_start(bk_sb[:], bk[:, :])

            pending_out = []

            def out_qh(AT16, t0, q0):
                # output projection for one q-half: 8 ot x 1 [128, 1024] tile
                for ot in range(D_MODEL // P):
                    psy = psS.tile([P, 1024], F32, tag="pss")
                    for tj in range(2):
                        c0 = q0 + tj * 512
                        nc.tensor.matmul(
                            psy[:, tj * 512:(tj + 1) * 512],
                            wo_sb[:, ot * P:(ot + 1) * P],
                            AT16[:, c0:c0 + 512],
                            start=True,
                            stop=True,
                        )
                    yst = outp.tile([P, 1024], F16, tag="yst")
                    if ot % 2 == 0:
                        nc.vector.tensor_copy(yst[:], psy[:])
                    else:
                        nc.scalar.copy(yst[:], psy[:])
                    nc.sync.dma_start(
                        yT[ot * P:(ot + 1) * P, t0 + q0:t0 + q0 + 1024],
                        yst[:],
                    )

            for bi in range(b):
                t0 = bi * s

                # ---- phase A: q/k/v projections ----
                qTb = stg.tile([HD, s], F16, tag="qTb")
                kTb = stg.tile([HD, s], F16, tag="kTb")
                vTb = stg.tile([HD, s], F16, tag="vTb")
                for t in range(n_tc):
                    c0 = t0 + t * 512
                    xt = xin.tile([P, dmc, 512], F16, tag="xt")
                    nc.sync.dma_start(xt[:], xT[:, :, c0:c0 + 512])
                    for w_sb, b_sb, dst in (
                        (wq_sb, bq_sb, qTb),
                        (wk_sb, bk_sb, kTb),
                        (wv_sb, None, vTb),
                    ):
                        ps = psA.tile([P, 512], F32, tag="psA")
                        for c in range(dmc):
                            nc.tensor.matmul(
                                ps[:],
                                w_sb[:, c, :],
                                xt[:, c, :],
                                start=(c == 0),
                                stop=(c == dmc - 1),
                            )
                        sl = dst[:, t * 512:(t + 1) * 512]
                        if b_sb is None:
                            nc.vector.tensor_copy(sl, ps[:])
                        else:
                            nc.vector.tensor_scalar_add(sl, ps[:], b_sb[:])

                # ---- phase B: v8 token-major with ones column ----
                # v8[p, kt, h, d] = v[token kt*128+p, head h dim d]; d=64 -> 1
                v8 = vp.tile([P, n_kt, HPC, D_HEAD + 1], F16, tag="v8")
                nc.vector.memset(v8[:, :, :, D_HEAD], 1.0)
                for ch in range(n_kt):
                    vps_f = psA.tile([P, 512], F32, tag="psA")
                    vps = vps_f[:, 0:P // 2].bitcast(F16)
                    nc.tensor.transpose(
                        vps, vTb[:, ch * P:(ch + 1) * P], ident16[:])
                    nc.vector.tensor_copy(
                        v8[:, ch, :, 0:D_HEAD],
                        vps.rearrange("p (h d) -> p h d", d=D_HEAD),
                    )

                # ---- phase C: attention ----
                AT16 = at16p.tile([HD, s], F16, tag="AT16")

                def norm_combo(pso, p0, q0):
                    # normalization: AT16 = pso[0:64] * (1 / sum)
                    # (broadcast the sum row via rank-1 PE matmul, then
                    # reciprocal on the [64, q] broadcast -- DVE time scales
                    # with free size only, so this costs the same as [1, q])
                    srow = smal.tile([1, 1024], F32R, tag="srow")
                    nc.vector.tensor_copy(srow[:], pso[D_HEAD:D_HEAD + 1, :])
                    bc = psS.tile([D_HEAD, 1024], F32, tag="pss")
                    for j in range(2):
                        nc.tensor.matmul(
                            bc[:, j * 512:(j + 1) * 512],
                            ones_a[:],
                            srow[:, j * 512:(j + 1) * 512],
                            start=True,
                            stop=True,
                        )
                    rinv = smal.tile([D_HEAD, 1024], F32, tag="rinv")
                    nc.vector.reciprocal_approx_fast(rinv[:], bc[:])
                    nc.vector.tensor_mul(
                        AT16[p0:p0 + D_HEAD, q0:q0 + 1024],
                        pso[0:D_HEAD, :], rinv[:])

                # software-pipelined across kt AND combos: A@V runs two kt
                # behind scores, and the previous combo's normalization is
                # emitted after the next combo's first scores tile, so the
                # in-order PE queue always has ready matmul work
                pending_norm = None
                for qh in range(n_qh):
                    q0 = qh * 1024
                    for h in range(HPC):
                        p0 = 64 * h
                        pso = psO.tile([D_HEAD + 1, 1024], F32, tag="pso")
                        atts = []
                        for kt in range(n_kt + 2):
                            if kt < n_kt:
                                att = attp.tile([P, 1024], F16, tag="att")
                                pss = psS.tile([P, 1024], F32, tag="pss")
                                for j in range(2):
                                    nc.tensor.matmul(
                                        pss[:, j * 512:(j + 1) * 512],
                                        kTb[p0:p0 + D_HEAD,
                                            kt * P:(kt + 1) * P],
                                        qTb[p0:p0 + D_HEAD,
                                            q0 + j * 512:q0 + (j + 1) * 512],
                                        start=True,
                                        stop=True,
                                    )
                                if (exp_idx % dve_den) < dve_num:
                                    # Schraudolph exp: fp16 bits ~ a*psum + b
                                    nc.vector.tensor_scalar(
                                        att[:].bitcast(I16),
                                        pss[:],
                                        A16,
                                        B16,
                                        op0=ALU.mult,
                                        op1=ALU.add,
                                    )
                                else:
                                    nc.scalar.activation(
                                        att[:], pss[:], AF.Exp,
                                        bias=ebias[:], scale=ACT_SCALE,
                                    )
                                exp_idx += 1
                                atts.append(att)
                            if kt == 1 and pending_norm is not None:
                                # deferred one tile further so the srow DVE
                                # round-trip hides under two scores tiles
                                norm_combo(*pending_norm)
                                pending_norm = None
                            if kt == 4 and pending_out:
                                out_qh(*pending_out.pop(0))
                            if kt >= 2:
                                ki = kt - 2
                                for j in range(2):
                                    nc.tensor.matmul(
                                        pso[:, j * 512:(j + 1) * 512],
                                        v8[:, ki, h, :],
                                        atts[ki][:, j * 512:(j + 1) * 512],
                                        start=(ki == 0),
                                        stop=(ki == n_kt - 1),
                                    )
                        pending_norm = (pso, p0, q0)
                        if h == HPC - 1:
                            pending_out.append((AT16, t0, q0))
                if pending_norm is not None:
                    norm_combo(*pending_norm)
                    pending_norm = None

            while pending_out:
                out_qh(*pending_out.pop(0))
    nc.compile()
    return nc


def host_inputs(inputs, Wq, bq, Wk, bk, Wv, bv, Wo, bo):
    """Prepare per-core input maps (fp16 host-side conversions)."""
    b, s, dm = inputs.shape
    tok = b * s
    dmc = dm // 128
    x2 = np.asarray(inputs, np.float32).reshape(tok, dmc, 128)
    xT16 = np.ascontiguousarray(x2.transpose(2, 1, 0)).astype(F16NP)

    def wprep(W, sl, scale):  # [hd, dm] slice -> [128, dmc, hd] fp16
        w = (np.asarray(W, np.float32)[sl, :] * scale).T  # [dm, hd]
        return np.ascontiguousarray(
            w.reshape(dmc, 128, HD).transpose(1, 0, 2)).astype(F16NP)

    in_maps = []
    for c in range(N_CORES):
        sl = slice(c * HD, (c + 1) * HD)
        wo_c = np.ascontiguousarray(
            np.asarray(Wo, np.float32)[:, sl].T).astype(F16NP)  # [128, dm]
        in_maps.append({
            "xT": xT16,
            "wq": wprep(Wq, sl, SQ),
            "wk": wprep(Wk, sl, SQ),
            "wv": wprep(Wv, sl, 1.0),
            "wo": wo_c,
            "bq": np.ascontiguousarray(
                (np.asarray(bq, np.float32)[sl] * SQ).reshape(HD, 1)),
            "bk": np.ascontiguousarray(
                (np.asarray(bk, np.float32)[sl] * SQ).reshape(HD, 1)),
        })
    return in_maps


_NC_CACHE = {}


def _get_nc(b, s):
    key = (b, s)
    if key not in _NC_CACHE:
        _NC_CACHE[key] = build_mha(b=b, s=s)
    return _NC_CACHE[key]


def kernel(inputs, Wq, bq, Wk, bk, Wv, bv, Wo, bo):
    inputs = np.asarray(inputs, dtype=np.float32)
    b, s, dm = inputs.shape

    in_maps = host_inputs(inputs, Wq, bq, Wk, bk, Wv, bv, Wo, bo)
    nc = _get_nc(b, s)
    res = run_bass_kernel_spmd(
        nc, in_maps, core_ids=list(range(N_CORES)), trace=_TRACE
    )
    acc = res.results[0]["yT"].astype(np.float32)
    for c in range(1, N_CORES):
        acc += res.results[c]["yT"].astype(np.float32)
    bo_eff = (np.asarray(bo, np.float64)
              + np.asarray(bv, np.float64) @ np.asarray(Wo, np.float64).T)
    out = acc.T + bo_eff[None, :].astype(np.float32)
    if _TRACE:
        kernel.last_results = res
    return out.reshape(b, s, dm).astype(np.float32)

